# revision 3
# baseline (speedup 1.0000x reference)
"""GORU cell kernel for Trainium2, data-parallel over batch on 8 NeuronCores.

Reference computation (B=8192, IN=D=1024, CAP=10):
    Ux = x @ U;  U_cx, U_rx, U_gx = split(Ux)
    r = sigmoid(U_rx + state @ W_r + bias_r)
    g = sigmoid(U_gx + state @ W_g + bias_g)
    h = butterfly_rotate(state, theta)          # 10 elementwise stages
    pre = r * h + U_cx
    c = sign(pre) * relu(|pre| + 0.001 + bias_c)
    out = g * state + (1 - g) * c

The butterfly is linear in `state`: each stage is h @ M_i with M_i sparse
(2 nonzeros/column), so h = state @ R where R = M_0 @ ... @ M_9 is a dense
orthogonal matrix we materialize on the host by feeding the identity through
the stage loop. On-device the whole cell is then two fused matmul groups
  Z = [ x@U_c | x@U_r + s@W_r | x@U_g + s@W_g | s@R ]
(PSUM-accumulated, bf16 inputs / fp32 accumulate) plus cheap elementwise
epilogue on the scalar/vector engines, all in feature-major layout so the
per-feature biases are per-partition scalars.
"""

import math

import numpy as np

# ---------------------------------------------------------------------------
# Problem constants (hardcoded; kernel.py must be self-contained)
# ---------------------------------------------------------------------------
B = 8192
D = 1024
IN = 1024
CAP = int(math.log2(D))  # 10
NCORES = 8
BC = B // NCORES  # 1024 batch rows per core
P = 128
KO = D // P  # 8 contraction chunks of 128
MO = D // P  # 8 output-feature chunks of 128
NF = 512  # moving free dim per matmul (one PSUM bank of fp32)
NT = BC // NF  # 2 batch chunks per core


def _gen_indices(s):
    """Static FFT-butterfly index lists (identical to the reference)."""

    def ind_s(k):
        if k == 0:
            return [np.array([1, 0])]
        temp = np.arange(2**k)
        list0 = [np.concatenate([temp + 2**k, temp])]
        list1 = ind_s(k - 1)
        for i in range(k):
            list0.append(np.concatenate([list1[i], list1[i] + 2**k]))
        return list0

    t = ind_s(int(math.log2(s // 2)))
    cap = int(math.log2(s))
    ind_exe = [np.asarray(t[i], dtype=np.int32) for i in range(cap)]
    ind_param = []
    for i in range(cap):
        ind = np.concatenate(
            [np.arange(0, s, 2**i) + j for j in range(2**i)]
        ).astype(np.int32)
        ind_param.append(ind)
    return ind_exe, ind_param


IND_EXE, IND_PARAM = _gen_indices(D)


def _butterfly_matrix(theta: np.ndarray) -> np.ndarray:
    """Dense [D, D] matrix R with butterfly(state) == state @ R."""
    theta = np.asarray(theta, np.float32)
    cos_list = np.concatenate([np.cos(theta), np.cos(theta)], axis=1)
    sin_list = np.concatenate([np.sin(theta), -np.sin(theta)], axis=1)
    h = np.eye(D, dtype=np.float32)
    for i in range(CAP):
        v1 = cos_list[i][IND_PARAM[i]]
        v2 = sin_list[i][IND_PARAM[i]]
        h = h * v1 + (h * v2)[:, IND_EXE[i]]
    return h


# ---------------------------------------------------------------------------
# Bass program (built once, reused across calls)
# ---------------------------------------------------------------------------
_NC_CACHE = None


def _build_bass():
    global _NC_CACHE
    if _NC_CACHE is not None:
        return _NC_CACHE

    try:
        import concourse.bacc as bacc
    except ImportError:
        import sys

        sys.path.insert(0, "/opt/trn_rl_repo")
        import concourse.bacc as bacc
    import concourse.mybir as mybir
    from concourse.tile import TileContext

    f32 = mybir.dt.float32
    bf16 = mybir.dt.bfloat16
    AF = mybir.ActivationFunctionType
    OP = mybir.AluOpType

    # Bacc (not plain Bass): its compile() pass splits multi-semaphore waits
    # into EventSemaphore prefixes — trn2 ISA allows only one wait per
    # compute instruction, and Tile freely assigns two.
    nc = bacc.Bacc()

    # Per-core inputs, pre-swizzled on the host so each DMA is contiguous
    # per partition.
    # Activations: [p, ko, n] with element = act[batch n, feature ko*128+p].
    xT = nc.dram_tensor("xT", [P, KO, BC], bf16, kind="ExternalInput")
    sT = nc.dram_tensor("sT", [P, KO, BC], bf16, kind="ExternalInput")
    sTf = nc.dram_tensor("sTf", [P, MO, BC], f32, kind="ExternalInput")
    # Weights: [p, block, m, ko, j] = W[ko*128+p, block*1024 + m*128 + j].
    wU = nc.dram_tensor("wU", [P, 3, MO, KO, P], bf16, kind="ExternalInput")
    wS = nc.dram_tensor("wS", [P, 3, MO, KO, P], bf16, kind="ExternalInput")
    # Per-feature vectors: [p, m] = vec[m*128 + p].
    br = nc.dram_tensor("br", [P, MO], f32, kind="ExternalInput")
    bg = nc.dram_tensor("bg", [P, MO], f32, kind="ExternalInput")
    bk = nc.dram_tensor("bk", [P, MO], f32, kind="ExternalInput")  # 0.001+bias_c
    # Output, feature-major: [feature, batch].
    outT = nc.dram_tensor("outT", [D, BC], f32, kind="ExternalOutput")

    with TileContext(nc) as tc:
        with (
            tc.tile_pool(name="acts", bufs=1) as acts,
            tc.tile_pool(name="consts", bufs=1) as consts,
            tc.tile_pool(name="wpool", bufs=2) as wpool,
            tc.tile_pool(name="psum", bufs=2, space="PSUM") as psum,
            tc.tile_pool(name="work", bufs=3) as work,
        )        :
            # Resident activations; per-ko DMAs so matmuls can start early.
            xt = acts.tile([P, KO, BC], bf16)
            st = acts.tile([P, KO, BC], bf16)
            stf = acts.tile([P, MO, BC], f32)
            for ko in range(KO):
                nc.sync.dma_start(out=xt[:, ko], in_=xT[:, ko])
            for ko in range(KO):
                nc.sync.dma_start(out=st[:, ko], in_=sT[:, ko])
            for ko in range(KO):
                nc.sync.dma_start(out=stf[:, ko], in_=sTf[:, ko])

            brt = consts.tile([P, MO], f32)
            bgt = consts.tile([P, MO], f32)
            bkt = consts.tile([P, MO], f32)
            nc.sync.dma_start(out=brt[:], in_=br[:])
            nc.sync.dma_start(out=bgt[:], in_=bg[:])
            nc.sync.dma_start(out=bkt[:], in_=bk[:])

            for m in range(MO):
                # Stream this m-chunk's six weight sets ([128, KO, 128] each).
                uc = wpool.tile([P, KO, P], bf16, tag="uc")
                ur = wpool.tile([P, KO, P], bf16, tag="ur")
                ug = wpool.tile([P, KO, P], bf16, tag="ug")
                wr = wpool.tile([P, KO, P], bf16, tag="wr")
                wg = wpool.tile([P, KO, P], bf16, tag="wg")
                rr = wpool.tile([P, KO, P], bf16, tag="rr")
                nc.sync.dma_start(out=uc[:], in_=wU[:, 0, m])
                nc.sync.dma_start(out=ur[:], in_=wU[:, 1, m])
                nc.sync.dma_start(out=ug[:], in_=wU[:, 2, m])
                nc.sync.dma_start(out=wr[:], in_=wS[:, 0, m])
                nc.sync.dma_start(out=wg[:], in_=wS[:, 1, m])
                nc.sync.dma_start(out=rr[:], in_=wS[:, 2, m])

                for n in range(NT):
                    ns = slice(n * NF, (n + 1) * NF)
                    pc = psum.tile([P, NF], f32, tag="pc")
                    pr = psum.tile([P, NF], f32, tag="pr")
                    pg = psum.tile([P, NF], f32, tag="pg")
                    ph = psum.tile([P, NF], f32, tag="ph")

                    for ko in range(KO):
                        nc.tensor.matmul(
                            pc[:], uc[:, ko], xt[:, ko, ns],
                            start=(ko == 0), stop=(ko == KO - 1),
                        )
                    for ko in range(KO):
                        nc.tensor.matmul(
                            pr[:], ur[:, ko], xt[:, ko, ns],
                            start=(ko == 0), stop=False,
                        )
                    for ko in range(KO):
                        nc.tensor.matmul(
                            pr[:], wr[:, ko], st[:, ko, ns],
                            start=False, stop=(ko == KO - 1),
                        )
                    for ko in range(KO):
                        nc.tensor.matmul(
                            pg[:], ug[:, ko], xt[:, ko, ns],
                            start=(ko == 0), stop=False,
                        )
                    for ko in range(KO):
                        nc.tensor.matmul(
                            pg[:], wg[:, ko], st[:, ko, ns],
                            start=False, stop=(ko == KO - 1),
                        )
                    for ko in range(KO):
                        nc.tensor.matmul(
                            ph[:], rr[:, ko], st[:, ko, ns],
                            start=(ko == 0), stop=(ko == KO - 1),
                        )

                    # Epilogue (feature-major: per-feature vectors are
                    # per-partition scalars).
                    rt = work.tile([P, NF], f32, tag="rt")
                    nc.scalar.activation(
                        rt[:], pr[:], AF.Sigmoid, bias=brt[:, m : m + 1]
                    )
                    gt = work.tile([P, NF], f32, tag="gt")
                    nc.scalar.activation(
                        gt[:], pg[:], AF.Sigmoid, bias=bgt[:, m : m + 1]
                    )
                    pre = work.tile([P, NF], f32, tag="pre")
                    nc.vector.tensor_mul(pre[:], rt[:], ph[:])
                    nc.vector.tensor_add(pre[:], pre[:], pc[:])

                    sgn = work.tile([P, NF], f32, tag="sgn")
                    nc.scalar.activation(sgn[:], pre[:], AF.Sign)
                    ab = work.tile([P, NF], f32, tag="ab")
                    nc.scalar.activation(ab[:], pre[:], AF.Abs)
                    t1 = work.tile([P, NF], f32, tag="t1")
                    nc.vector.tensor_scalar(
                        t1[:], ab[:], bkt[:, m : m + 1], 0.0, OP.add, OP.max
                    )
                    ct = work.tile([P, NF], f32, tag="ct")
                    nc.vector.tensor_mul(ct[:], t1[:], sgn[:])

                    dt_ = work.tile([P, NF], f32, tag="dt")
                    nc.vector.tensor_sub(dt_[:], stf[:, m, ns], ct[:])
                    nc.vector.tensor_mul(dt_[:], gt[:], dt_[:])
                    ot = work.tile([P, NF], f32, tag="ot")
                    nc.vector.tensor_add(ot[:], dt_[:], ct[:])

                    nc.sync.dma_start(
                        out=outT[m * P : (m + 1) * P, ns], in_=ot[:]
                    )

    nc.finalize()  # Bacc.finalize → compile(): wait splitting, reg alloc, DCE
    _NC_CACHE = nc
    return nc


# ---------------------------------------------------------------------------
# Host-side sharding / swizzling
# ---------------------------------------------------------------------------
def _swizzle_w(w: np.ndarray, bf16) -> np.ndarray:
    """[K=1024, 3072] -> [p, block, m, ko, j] bf16."""
    w = np.asarray(w, np.float32).reshape(KO, P, 3, MO, P)
    return np.ascontiguousarray(w.transpose(1, 2, 3, 0, 4)).astype(bf16)


def _swizzle_act(a: np.ndarray, dtype) -> np.ndarray:
    """[BC, 1024] -> [p, ko, n]."""
    at = np.asarray(a, np.float32).T.reshape(KO, P, BC)
    return np.ascontiguousarray(at.transpose(1, 0, 2)).astype(dtype)


def _prepare_in_maps(inputs):
    import ml_dtypes

    bf16 = ml_dtypes.bfloat16

    x = np.asarray(inputs["x"], np.float32)
    state = np.asarray(inputs["state"], np.float32)
    theta = np.asarray(inputs["theta"], np.float32)
    U = np.asarray(inputs["U"], np.float32)
    W_r = np.asarray(inputs["W_r"], np.float32)
    W_g = np.asarray(inputs["W_g"], np.float32)
    bias_r = np.asarray(inputs["bias_r"], np.float32)
    bias_g = np.asarray(inputs["bias_g"], np.float32)
    bias_c = np.asarray(inputs["bias_c"], np.float32)

    R = _butterfly_matrix(theta)
    WS = np.concatenate([W_r, W_g, R], axis=1)  # [1024, 3072]

    wU_dev = _swizzle_w(U, bf16)
    wS_dev = _swizzle_w(WS, bf16)
    br_dev = np.ascontiguousarray(bias_r.reshape(MO, P).T)
    bg_dev = np.ascontiguousarray(bias_g.reshape(MO, P).T)
    bk_dev = np.ascontiguousarray((0.001 + bias_c).reshape(MO, P).T)

    in_maps = []
    for i in range(NCORES):
        rows = slice(i * BC, (i + 1) * BC)
        in_maps.append(
            {
                "xT": _swizzle_act(x[rows], bf16),
                "sT": _swizzle_act(state[rows], bf16),
                "sTf": _swizzle_act(state[rows], np.float32),
                "wU": wU_dev,
                "wS": wS_dev,
                "br": br_dev,
                "bg": bg_dev,
                "bk": bk_dev,
            }
        )
    return in_maps


def run(inputs, trace: bool = False):
    """Run the kernel; returns (out [8192, 1024] f32, BassKernelResults)."""
    nc = _build_bass()
    try:
        from concourse.bass_utils import run_bass_kernel_spmd
    except ImportError:
        import sys

        sys.path.insert(0, "/opt/trn_rl_repo")
        from concourse.bass_utils import run_bass_kernel_spmd

    in_maps = _prepare_in_maps(inputs)
    res = run_bass_kernel_spmd(
        nc, in_maps, core_ids=list(range(NCORES)), trace=trace
    )
    out = np.empty((B, D), np.float32)
    for i in range(NCORES):
        out[i * BC : (i + 1) * BC] = res.results[i]["outT"].T
    return out, res


def kernel(**inputs) -> np.ndarray:
    out, _ = run(inputs)
    return out


# revision 8
# speedup vs baseline: 1.0640x; 1.0640x over previous
"""GORU cell kernel for Trainium2, data-parallel over batch on 8 NeuronCores.

Reference computation (B=8192, IN=D=1024, CAP=10):
    Ux = x @ U;  U_cx, U_rx, U_gx = split(Ux)
    r = sigmoid(U_rx + state @ W_r + bias_r)
    g = sigmoid(U_gx + state @ W_g + bias_g)
    h = butterfly_rotate(state, theta)          # 10 elementwise stages
    pre = r * h + U_cx
    c = sign(pre) * relu(|pre| + 0.001 + bias_c)
    out = g * state + (1 - g) * c

The butterfly is linear in `state`: each stage is h @ M_i with M_i sparse
(2 nonzeros/column), so h = state @ R where R = M_0 @ ... @ M_9 is a dense
orthogonal matrix we materialize on the host by feeding the identity through
the stage loop. On-device the whole cell is then two fused matmul groups
  Z = [ x@U_c | x@U_r + s@W_r | x@U_g + s@W_g | s@R ]
(PSUM-accumulated, bf16 inputs / fp32 accumulate) plus cheap elementwise
epilogue on the scalar/vector engines, all in feature-major layout so the
per-feature biases are per-partition scalars.
"""

import math

import numpy as np

# ---------------------------------------------------------------------------
# Problem constants (hardcoded; kernel.py must be self-contained)
# ---------------------------------------------------------------------------
B = 8192
D = 1024
IN = 1024
CAP = int(math.log2(D))  # 10
NCORES = 8
BC = B // NCORES  # 1024 batch rows per core
P = 128
KO = D // P  # 8 contraction chunks of 128
MO = D // P  # 8 output-feature chunks of 128
NF = 512  # moving free dim per matmul (one PSUM bank of fp32)
NT = BC // NF  # 2 batch chunks per core


def _gen_indices(s):
    """Static FFT-butterfly index lists (identical to the reference)."""

    def ind_s(k):
        if k == 0:
            return [np.array([1, 0])]
        temp = np.arange(2**k)
        list0 = [np.concatenate([temp + 2**k, temp])]
        list1 = ind_s(k - 1)
        for i in range(k):
            list0.append(np.concatenate([list1[i], list1[i] + 2**k]))
        return list0

    t = ind_s(int(math.log2(s // 2)))
    cap = int(math.log2(s))
    ind_exe = [np.asarray(t[i], dtype=np.int32) for i in range(cap)]
    ind_param = []
    for i in range(cap):
        ind = np.concatenate(
            [np.arange(0, s, 2**i) + j for j in range(2**i)]
        ).astype(np.int32)
        ind_param.append(ind)
    return ind_exe, ind_param


IND_EXE, IND_PARAM = _gen_indices(D)


def _butterfly_matrix(theta: np.ndarray) -> np.ndarray:
    """Dense [D, D] matrix R with butterfly(state) == state @ R."""
    theta = np.asarray(theta, np.float32)
    cos_list = np.concatenate([np.cos(theta), np.cos(theta)], axis=1)
    sin_list = np.concatenate([np.sin(theta), -np.sin(theta)], axis=1)
    h = np.eye(D, dtype=np.float32)
    for i in range(CAP):
        v1 = cos_list[i][IND_PARAM[i]]
        v2 = sin_list[i][IND_PARAM[i]]
        h = h * v1 + (h * v2)[:, IND_EXE[i]]
    return h


# ---------------------------------------------------------------------------
# Bass program (built once, reused across calls)
# ---------------------------------------------------------------------------
_NC_CACHE = {}


def _build_bass(fast_modrelu: bool):
    """fast_modrelu: when every 0.001+bias_c entry is > 0, the relu in the
    modReLU is the identity and c = pre + k*sign(pre) (one fused DVE op)."""
    global _NC_CACHE
    if fast_modrelu in _NC_CACHE:
        return _NC_CACHE[fast_modrelu]

    try:
        import concourse.bacc as bacc
    except ImportError:
        import sys

        sys.path.insert(0, "/opt/trn_rl_repo")
        import concourse.bacc as bacc
    import concourse.mybir as mybir
    from concourse.tile import TileContext

    f32 = mybir.dt.float32
    bf16 = mybir.dt.bfloat16
    AF = mybir.ActivationFunctionType
    OP = mybir.AluOpType

    # Bacc (not plain Bass): its compile() pass splits multi-semaphore waits
    # into EventSemaphore prefixes — trn2 ISA allows only one wait per
    # compute instruction, and Tile freely assigns two.
    nc = bacc.Bacc()

    # Per-core inputs, pre-swizzled on the host so each DMA is contiguous
    # per partition.
    # Activations: [p, ko, n] with element = act[batch n, feature ko*128+p].
    xT = nc.dram_tensor("xT", [P, KO, BC], bf16, kind="ExternalInput")
    sT = nc.dram_tensor("sT", [P, KO, BC], bf16, kind="ExternalInput")
    sTf = nc.dram_tensor("sTf", [P, MO, BC], f32, kind="ExternalInput")
    # Weights: [p, block, m, ko, j] = W[ko*128+p, block*1024 + m*128 + j].
    wU = nc.dram_tensor("wU", [P, 3, MO, KO, P], bf16, kind="ExternalInput")
    wS = nc.dram_tensor("wS", [P, 3, MO, KO, P], bf16, kind="ExternalInput")
    # Per-feature vectors: [p, m] = vec[m*128 + p].
    br = nc.dram_tensor("br", [P, MO], f32, kind="ExternalInput")
    bg = nc.dram_tensor("bg", [P, MO], f32, kind="ExternalInput")
    bk = nc.dram_tensor("bk", [P, MO], f32, kind="ExternalInput")  # 0.001+bias_c
    # Output, feature-major: [feature, batch].
    outT = nc.dram_tensor("outT", [D, BC], f32, kind="ExternalOutput")

    with TileContext(nc) as tc:
        with (
            tc.tile_pool(name="acts", bufs=1) as acts,
            tc.tile_pool(name="consts", bufs=1) as consts,
            tc.tile_pool(name="wpool", bufs=2) as wpool,
            tc.tile_pool(name="psum", bufs=2, space="PSUM") as psum,
            tc.tile_pool(name="work", bufs=3) as work,
        )        :
            # DMA emission order == wire order (one queue). The first matmul
            # needs only uc(m=0) + xT, so those go first; stf (only needed by
            # the first epilogue, ~40us in) goes last.
            def load_w(name, src):
                t = wpool.tile([P, KO, P], bf16, tag=name)
                nc.sync.dma_start(out=t[:], in_=src)
                return t

            xt = acts.tile([P, KO, BC], bf16)
            st = acts.tile([P, KO, BC], bf16)
            stf = acts.tile([P, MO, BC], f32)

            w0 = {"uc": load_w("uc", wU[:, 0, 0])}
            for ko in range(KO):
                nc.sync.dma_start(out=xt[:, ko], in_=xT[:, ko])
            w0["ur"] = load_w("ur", wU[:, 1, 0])
            w0["ug"] = load_w("ug", wU[:, 2, 0])
            for ko in range(KO):
                nc.sync.dma_start(out=st[:, ko], in_=sT[:, ko])
            w0["wr"] = load_w("wr", wS[:, 0, 0])
            w0["wg"] = load_w("wg", wS[:, 1, 0])
            w0["rr"] = load_w("rr", wS[:, 2, 0])

            brt = consts.tile([P, MO], f32)
            bgt = consts.tile([P, MO], f32)
            bkt = consts.tile([P, MO], f32)
            nc.sync.dma_start(out=brt[:], in_=br[:])
            nc.sync.dma_start(out=bgt[:], in_=bg[:])
            nc.sync.dma_start(out=bkt[:], in_=bk[:])
            for ko in range(KO):
                nc.sync.dma_start(out=stf[:, ko], in_=sTf[:, ko])

            for m in range(MO):
                if m == 0:
                    uc, ur, ug = w0["uc"], w0["ur"], w0["ug"]
                    wr, wg, rr = w0["wr"], w0["wg"], w0["rr"]
                else:
                    uc = load_w("uc", wU[:, 0, m])
                    ur = load_w("ur", wU[:, 1, m])
                    ug = load_w("ug", wU[:, 2, m])
                    wr = load_w("wr", wS[:, 0, m])
                    wg = load_w("wg", wS[:, 1, m])
                    rr = load_w("rr", wS[:, 2, m])

                for n in range(NT):
                    ns = slice(n * NF, (n + 1) * NF)
                    pc = psum.tile([P, NF], f32, tag="pc")
                    pr = psum.tile([P, NF], f32, tag="pr")
                    pg = psum.tile([P, NF], f32, tag="pg")
                    ph = psum.tile([P, NF], f32, tag="ph")

                    for ko in range(KO):
                        nc.tensor.matmul(
                            pc[:], uc[:, ko], xt[:, ko, ns],
                            start=(ko == 0), stop=(ko == KO - 1),
                        )
                    for ko in range(KO):
                        nc.tensor.matmul(
                            pr[:], ur[:, ko], xt[:, ko, ns],
                            start=(ko == 0), stop=False,
                        )
                    for ko in range(KO):
                        nc.tensor.matmul(
                            pr[:], wr[:, ko], st[:, ko, ns],
                            start=False, stop=(ko == KO - 1),
                        )
                    for ko in range(KO):
                        nc.tensor.matmul(
                            pg[:], ug[:, ko], xt[:, ko, ns],
                            start=(ko == 0), stop=False,
                        )
                    for ko in range(KO):
                        nc.tensor.matmul(
                            pg[:], wg[:, ko], st[:, ko, ns],
                            start=False, stop=(ko == KO - 1),
                        )
                    for ko in range(KO):
                        nc.tensor.matmul(
                            ph[:], rr[:, ko], st[:, ko, ns],
                            start=(ko == 0), stop=(ko == KO - 1),
                        )

                    # Epilogue (feature-major: per-feature vectors are
                    # per-partition scalars).
                    rt = work.tile([P, NF], f32, tag="rt")
                    nc.scalar.activation(
                        rt[:], pr[:], AF.Sigmoid, bias=brt[:, m : m + 1]
                    )
                    gt = work.tile([P, NF], f32, tag="gt")
                    nc.scalar.activation(
                        gt[:], pg[:], AF.Sigmoid, bias=bgt[:, m : m + 1]
                    )
                    pre = work.tile([P, NF], f32, tag="pre")
                    nc.vector.tensor_mul(pre[:], rt[:], ph[:])
                    nc.vector.tensor_add(pre[:], pre[:], pc[:])

                    sgn = work.tile([P, NF], f32, tag="sgn")
                    nc.scalar.activation(sgn[:], pre[:], AF.Sign)
                    ct = work.tile([P, NF], f32, tag="ct")
                    if fast_modrelu:
                        # c = pre + k*sign(pre)   (valid because k > 0)
                        nc.vector.scalar_tensor_tensor(
                            ct[:], sgn[:], bkt[:, m : m + 1], pre[:],
                            OP.mult, OP.add,
                        )
                    else:
                        ab = work.tile([P, NF], f32, tag="ab")
                        nc.scalar.activation(ab[:], pre[:], AF.Abs)
                        t1 = work.tile([P, NF], f32, tag="t1")
                        nc.vector.tensor_scalar(
                            t1[:], ab[:], bkt[:, m : m + 1], 0.0, OP.add, OP.max
                        )
                        nc.vector.tensor_mul(ct[:], t1[:], sgn[:])

                    dt_ = work.tile([P, NF], f32, tag="dt")
                    nc.vector.tensor_sub(dt_[:], stf[:, m, ns], ct[:])
                    nc.vector.tensor_mul(dt_[:], gt[:], dt_[:])
                    ot = work.tile([P, NF], f32, tag="ot")
                    nc.vector.tensor_add(ot[:], dt_[:], ct[:])

                    nc.sync.dma_start(
                        out=outT[m * P : (m + 1) * P, ns], in_=ot[:]
                    )

    nc.finalize()  # Bacc.finalize → compile(): wait splitting, reg alloc, DCE
    _NC_CACHE[fast_modrelu] = nc
    return nc


# ---------------------------------------------------------------------------
# Host-side sharding / swizzling
# ---------------------------------------------------------------------------
def _swizzle_w(w: np.ndarray, bf16) -> np.ndarray:
    """[K=1024, 3072] -> [p, block, m, ko, j] bf16."""
    w = np.asarray(w, np.float32).reshape(KO, P, 3, MO, P)
    return np.ascontiguousarray(w.transpose(1, 2, 3, 0, 4)).astype(bf16)


def _swizzle_act(a: np.ndarray, dtype) -> np.ndarray:
    """[BC, 1024] -> [p, ko, n]."""
    at = np.asarray(a, np.float32).T.reshape(KO, P, BC)
    return np.ascontiguousarray(at.transpose(1, 0, 2)).astype(dtype)


def _prepare_in_maps(inputs):
    import ml_dtypes

    bf16 = ml_dtypes.bfloat16

    x = np.asarray(inputs["x"], np.float32)
    state = np.asarray(inputs["state"], np.float32)
    theta = np.asarray(inputs["theta"], np.float32)
    U = np.asarray(inputs["U"], np.float32)
    W_r = np.asarray(inputs["W_r"], np.float32)
    W_g = np.asarray(inputs["W_g"], np.float32)
    bias_r = np.asarray(inputs["bias_r"], np.float32)
    bias_g = np.asarray(inputs["bias_g"], np.float32)
    bias_c = np.asarray(inputs["bias_c"], np.float32)

    R = _butterfly_matrix(theta)
    WS = np.concatenate([W_r, W_g, R], axis=1)  # [1024, 3072]

    wU_dev = _swizzle_w(U, bf16)
    wS_dev = _swizzle_w(WS, bf16)
    br_dev = np.ascontiguousarray(bias_r.reshape(MO, P).T)
    bg_dev = np.ascontiguousarray(bias_g.reshape(MO, P).T)
    bk_dev = np.ascontiguousarray((0.001 + bias_c).reshape(MO, P).T)

    in_maps = []
    for i in range(NCORES):
        rows = slice(i * BC, (i + 1) * BC)
        in_maps.append(
            {
                "xT": _swizzle_act(x[rows], bf16),
                "sT": _swizzle_act(state[rows], bf16),
                "sTf": _swizzle_act(state[rows], np.float32),
                "wU": wU_dev,
                "wS": wS_dev,
                "br": br_dev,
                "bg": bg_dev,
                "bk": bk_dev,
            }
        )
    return in_maps


def run(inputs, trace: bool = False):
    """Run the kernel; returns (out [8192, 1024] f32, BassKernelResults)."""
    fast = bool(
        np.all(0.001 + np.asarray(inputs["bias_c"], np.float32) > 0)
    )
    nc = _build_bass(fast)
    try:
        from concourse.bass_utils import run_bass_kernel_spmd
    except ImportError:
        import sys

        sys.path.insert(0, "/opt/trn_rl_repo")
        from concourse.bass_utils import run_bass_kernel_spmd

    in_maps = _prepare_in_maps(inputs)
    res = run_bass_kernel_spmd(
        nc, in_maps, core_ids=list(range(NCORES)), trace=trace
    )
    out = np.empty((B, D), np.float32)
    for i in range(NCORES):
        out[i * BC : (i + 1) * BC] = res.results[i]["outT"].T
    return out, res


def kernel(**inputs) -> np.ndarray:
    out, _ = run(inputs)
    return out


# revision 11
# speedup vs baseline: 1.1176x; 1.0504x over previous
"""GORU cell kernel for Trainium2, data-parallel over batch on 8 NeuronCores.

Reference computation (B=8192, IN=D=1024, CAP=10):
    Ux = x @ U;  U_cx, U_rx, U_gx = split(Ux)
    r = sigmoid(U_rx + state @ W_r + bias_r)
    g = sigmoid(U_gx + state @ W_g + bias_g)
    h = butterfly_rotate(state, theta)          # 10 elementwise stages
    pre = r * h + U_cx
    c = sign(pre) * relu(|pre| + 0.001 + bias_c)
    out = g * state + (1 - g) * c

The butterfly is linear in `state`: each stage is h @ M_i with M_i sparse
(2 nonzeros/column), so h = state @ R where R = M_0 @ ... @ M_9 is a dense
orthogonal matrix we materialize on the host by feeding the identity through
the stage loop. On-device the whole cell is then two fused matmul groups
  Z = [ x@U_c | x@U_r + s@W_r | x@U_g + s@W_g | s@R ]
(PSUM-accumulated, bf16 inputs / fp32 accumulate) plus cheap elementwise
epilogue on the scalar/vector engines, all in feature-major layout so the
per-feature biases are per-partition scalars.
"""

import math

import numpy as np

# ---------------------------------------------------------------------------
# Problem constants (hardcoded; kernel.py must be self-contained)
# ---------------------------------------------------------------------------
B = 8192
D = 1024
IN = 1024
CAP = int(math.log2(D))  # 10
NCORES = 8
BC = B // NCORES  # 1024 batch rows per core
P = 128
KO = D // P  # 8 contraction chunks of 128
MO = D // P  # 8 output-feature chunks of 128
NF = 512  # moving free dim per matmul (one PSUM bank of fp32)
NT = BC // NF  # 2 batch chunks per core


def _gen_indices(s):
    """Static FFT-butterfly index lists (identical to the reference)."""

    def ind_s(k):
        if k == 0:
            return [np.array([1, 0])]
        temp = np.arange(2**k)
        list0 = [np.concatenate([temp + 2**k, temp])]
        list1 = ind_s(k - 1)
        for i in range(k):
            list0.append(np.concatenate([list1[i], list1[i] + 2**k]))
        return list0

    t = ind_s(int(math.log2(s // 2)))
    cap = int(math.log2(s))
    ind_exe = [np.asarray(t[i], dtype=np.int32) for i in range(cap)]
    ind_param = []
    for i in range(cap):
        ind = np.concatenate(
            [np.arange(0, s, 2**i) + j for j in range(2**i)]
        ).astype(np.int32)
        ind_param.append(ind)
    return ind_exe, ind_param


IND_EXE, IND_PARAM = _gen_indices(D)


def _butterfly_matrix(theta: np.ndarray) -> np.ndarray:
    """Dense [D, D] matrix R with butterfly(state) == state @ R."""
    theta = np.asarray(theta, np.float32)
    cos_list = np.concatenate([np.cos(theta), np.cos(theta)], axis=1)
    sin_list = np.concatenate([np.sin(theta), -np.sin(theta)], axis=1)
    h = np.eye(D, dtype=np.float32)
    for i in range(CAP):
        v1 = cos_list[i][IND_PARAM[i]]
        v2 = sin_list[i][IND_PARAM[i]]
        h = h * v1 + (h * v2)[:, IND_EXE[i]]
    return h


# ---------------------------------------------------------------------------
# Bass program (built once, reused across calls)
# ---------------------------------------------------------------------------
_NC_CACHE = {}


def _build_bass(fast_modrelu: bool):
    """fast_modrelu: when every 0.001+bias_c entry is > 0, the relu in the
    modReLU is the identity and c = pre + k*sign(pre) (one fused DVE op)."""
    global _NC_CACHE
    if fast_modrelu in _NC_CACHE:
        return _NC_CACHE[fast_modrelu]

    try:
        import concourse.bacc as bacc
    except ImportError:
        import sys

        sys.path.insert(0, "/opt/trn_rl_repo")
        import concourse.bacc as bacc
    import concourse.mybir as mybir
    from concourse.tile import TileContext

    f32 = mybir.dt.float32
    bf16 = mybir.dt.bfloat16
    AF = mybir.ActivationFunctionType
    OP = mybir.AluOpType

    # Bacc (not plain Bass): its compile() pass splits multi-semaphore waits
    # into EventSemaphore prefixes — trn2 ISA allows only one wait per
    # compute instruction, and Tile freely assigns two.
    nc = bacc.Bacc()

    # Per-core inputs, pre-swizzled on the host so each DMA is contiguous
    # per partition.
    # Activations: [p, ko, n] with element = act[batch n, feature ko*128+p].
    xT = nc.dram_tensor("xT", [P, KO, BC], bf16, kind="ExternalInput")
    sT = nc.dram_tensor("sT", [P, KO, BC], bf16, kind="ExternalInput")
    sTf = nc.dram_tensor("sTf", [P, MO, BC], f32, kind="ExternalInput")
    # Weights: [p, block, m, ko, j] = W[ko*128+p, block*1024 + m*128 + j].
    wU = nc.dram_tensor("wU", [P, 3, MO, KO, P], bf16, kind="ExternalInput")
    wS = nc.dram_tensor("wS", [P, 3, MO, KO, P], bf16, kind="ExternalInput")
    # Per-feature vectors: [p, m] = vec[m*128 + p].
    br = nc.dram_tensor("br", [P, MO], f32, kind="ExternalInput")
    bg = nc.dram_tensor("bg", [P, MO], f32, kind="ExternalInput")
    bk = nc.dram_tensor("bk", [P, MO], f32, kind="ExternalInput")  # 0.001+bias_c
    # Output, feature-major: [feature, batch].
    outT = nc.dram_tensor("outT", [D, BC], f32, kind="ExternalOutput")

    with TileContext(nc) as tc:
        with (
            tc.tile_pool(name="acts", bufs=1) as acts,
            tc.tile_pool(name="consts", bufs=1) as consts,
            tc.tile_pool(name="wpool", bufs=3) as wpool,
            tc.tile_pool(name="psum", bufs=2, space="PSUM") as psum,
            tc.tile_pool(name="work", bufs=3) as work,
        )        :
            # DMA emission order == wire order (one queue). The first matmul
            # needs only uc(m=0) + xT, so those go first; stf (only needed by
            # the first epilogue, ~40us in) goes last.
            def load_w(name, src):
                t = wpool.tile([P, KO, P], bf16, tag=name)
                nc.sync.dma_start(out=t[:], in_=src)
                return t

            def load_w6(m):
                return {
                    "uc": load_w("uc", wU[:, 0, m]),
                    "ur": load_w("ur", wU[:, 1, m]),
                    "ug": load_w("ug", wU[:, 2, m]),
                    "wr": load_w("wr", wS[:, 0, m]),
                    "wg": load_w("wg", wS[:, 1, m]),
                    "rr": load_w("rr", wS[:, 2, m]),
                }

            xt = acts.tile([P, KO, BC], bf16)
            st = acts.tile([P, KO, BC], bf16)
            stf = acts.tile([P, MO, BC], f32)

            wts = {}
            wts[0] = {"uc": load_w("uc", wU[:, 0, 0])}
            for ko in range(KO):
                nc.sync.dma_start(out=xt[:, ko], in_=xT[:, ko])
            wts[0]["ur"] = load_w("ur", wU[:, 1, 0])
            wts[0]["ug"] = load_w("ug", wU[:, 2, 0])
            for ko in range(KO):
                nc.sync.dma_start(out=st[:, ko], in_=sT[:, ko])
            wts[0]["wr"] = load_w("wr", wS[:, 0, 0])
            wts[0]["wg"] = load_w("wg", wS[:, 1, 0])
            wts[0]["rr"] = load_w("rr", wS[:, 2, 0])
            wts[1] = load_w6(1)

            brt = consts.tile([P, MO], f32)
            bgt = consts.tile([P, MO], f32)
            bkt = consts.tile([P, MO], f32)
            nc.sync.dma_start(out=brt[:], in_=br[:])
            nc.sync.dma_start(out=bgt[:], in_=bg[:])
            nc.sync.dma_start(out=bkt[:], in_=bk[:])
            for ko in range(KO):
                nc.sync.dma_start(out=stf[:, ko], in_=sTf[:, ko])

            for m in range(MO):
                # Prefetch weights two m-iterations ahead (bufs=3 per tag:
                # in-use, loaded, loading).
                if m + 2 < MO:
                    wts[m + 2] = load_w6(m + 2)
                w = wts.pop(m)
                uc, ur, ug = w["uc"], w["ur"], w["ug"]
                wr, wg, rr = w["wr"], w["wg"], w["rr"]

                for n in range(NT):
                    ns = slice(n * NF, (n + 1) * NF)
                    pc = psum.tile([P, NF], f32, tag="pc")
                    pr = psum.tile([P, NF], f32, tag="pr")
                    pg = psum.tile([P, NF], f32, tag="pg")
                    ph = psum.tile([P, NF], f32, tag="ph")

                    # Block order h, r, c, g: the epilogue chain up to
                    # d = stf - c runs while the g-block matmuls stream, so
                    # only sigmoid(g) -> e -> out remains after the last MM.
                    for ko in range(KO):
                        nc.tensor.matmul(
                            ph[:], rr[:, ko], st[:, ko, ns],
                            start=(ko == 0), stop=(ko == KO - 1),
                        )
                    for ko in range(KO):
                        nc.tensor.matmul(
                            pr[:], ur[:, ko], xt[:, ko, ns],
                            start=(ko == 0), stop=False,
                        )
                    for ko in range(KO):
                        nc.tensor.matmul(
                            pr[:], wr[:, ko], st[:, ko, ns],
                            start=False, stop=(ko == KO - 1),
                        )
                    rt = work.tile([P, NF], f32, tag="rt")
                    nc.scalar.activation(
                        rt[:], pr[:], AF.Sigmoid, bias=brt[:, m : m + 1]
                    )
                    pre = work.tile([P, NF], f32, tag="pre")
                    nc.vector.tensor_mul(pre[:], rt[:], ph[:])

                    for ko in range(KO):
                        nc.tensor.matmul(
                            pc[:], uc[:, ko], xt[:, ko, ns],
                            start=(ko == 0), stop=(ko == KO - 1),
                        )
                    nc.vector.tensor_add(pre[:], pre[:], pc[:])
                    sgn = work.tile([P, NF], f32, tag="sgn")
                    nc.scalar.activation(sgn[:], pre[:], AF.Sign)
                    ct = work.tile([P, NF], f32, tag="ct")
                    if fast_modrelu:
                        # c = pre + k*sign(pre)   (valid because k > 0)
                        nc.vector.scalar_tensor_tensor(
                            ct[:], sgn[:], bkt[:, m : m + 1], pre[:],
                            OP.mult, OP.add,
                        )
                    else:
                        ab = work.tile([P, NF], f32, tag="ab")
                        nc.scalar.activation(ab[:], pre[:], AF.Abs)
                        t1 = work.tile([P, NF], f32, tag="t1")
                        nc.vector.tensor_scalar(
                            t1[:], ab[:], bkt[:, m : m + 1], 0.0, OP.add, OP.max
                        )
                        nc.vector.tensor_mul(ct[:], t1[:], sgn[:])
                    dt_ = work.tile([P, NF], f32, tag="dt")
                    nc.vector.tensor_sub(dt_[:], stf[:, m, ns], ct[:])

                    for ko in range(KO):
                        nc.tensor.matmul(
                            pg[:], ug[:, ko], xt[:, ko, ns],
                            start=(ko == 0), stop=False,
                        )
                    for ko in range(KO):
                        nc.tensor.matmul(
                            pg[:], wg[:, ko], st[:, ko, ns],
                            start=False, stop=(ko == KO - 1),
                        )
                    gt = work.tile([P, NF], f32, tag="gt")
                    nc.scalar.activation(
                        gt[:], pg[:], AF.Sigmoid, bias=bgt[:, m : m + 1]
                    )
                    nc.vector.tensor_mul(dt_[:], gt[:], dt_[:])
                    ot = work.tile([P, NF], f32, tag="ot")
                    nc.vector.tensor_add(ot[:], dt_[:], ct[:])

                    nc.sync.dma_start(
                        out=outT[m * P : (m + 1) * P, ns], in_=ot[:]
                    )

    nc.finalize()  # Bacc.finalize → compile(): wait splitting, reg alloc, DCE
    _NC_CACHE[fast_modrelu] = nc
    return nc


# ---------------------------------------------------------------------------
# Host-side sharding / swizzling
# ---------------------------------------------------------------------------
def _swizzle_w(w: np.ndarray, bf16) -> np.ndarray:
    """[K=1024, 3072] -> [p, block, m, ko, j] bf16."""
    w = np.asarray(w, np.float32).reshape(KO, P, 3, MO, P)
    return np.ascontiguousarray(w.transpose(1, 2, 3, 0, 4)).astype(bf16)


def _swizzle_act(a: np.ndarray, dtype) -> np.ndarray:
    """[BC, 1024] -> [p, ko, n]."""
    at = np.asarray(a, np.float32).T.reshape(KO, P, BC)
    return np.ascontiguousarray(at.transpose(1, 0, 2)).astype(dtype)


def _prepare_in_maps(inputs):
    import ml_dtypes

    bf16 = ml_dtypes.bfloat16

    x = np.asarray(inputs["x"], np.float32)
    state = np.asarray(inputs["state"], np.float32)
    theta = np.asarray(inputs["theta"], np.float32)
    U = np.asarray(inputs["U"], np.float32)
    W_r = np.asarray(inputs["W_r"], np.float32)
    W_g = np.asarray(inputs["W_g"], np.float32)
    bias_r = np.asarray(inputs["bias_r"], np.float32)
    bias_g = np.asarray(inputs["bias_g"], np.float32)
    bias_c = np.asarray(inputs["bias_c"], np.float32)

    R = _butterfly_matrix(theta)
    WS = np.concatenate([W_r, W_g, R], axis=1)  # [1024, 3072]

    wU_dev = _swizzle_w(U, bf16)
    wS_dev = _swizzle_w(WS, bf16)
    br_dev = np.ascontiguousarray(bias_r.reshape(MO, P).T)
    bg_dev = np.ascontiguousarray(bias_g.reshape(MO, P).T)
    bk_dev = np.ascontiguousarray((0.001 + bias_c).reshape(MO, P).T)

    in_maps = []
    for i in range(NCORES):
        rows = slice(i * BC, (i + 1) * BC)
        in_maps.append(
            {
                "xT": _swizzle_act(x[rows], bf16),
                "sT": _swizzle_act(state[rows], bf16),
                "sTf": _swizzle_act(state[rows], np.float32),
                "wU": wU_dev,
                "wS": wS_dev,
                "br": br_dev,
                "bg": bg_dev,
                "bk": bk_dev,
            }
        )
    return in_maps


def run(inputs, trace: bool = False):
    """Run the kernel; returns (out [8192, 1024] f32, BassKernelResults)."""
    fast = bool(
        np.all(0.001 + np.asarray(inputs["bias_c"], np.float32) > 0)
    )
    nc = _build_bass(fast)
    try:
        from concourse.bass_utils import run_bass_kernel_spmd
    except ImportError:
        import sys

        sys.path.insert(0, "/opt/trn_rl_repo")
        from concourse.bass_utils import run_bass_kernel_spmd

    in_maps = _prepare_in_maps(inputs)
    res = run_bass_kernel_spmd(
        nc, in_maps, core_ids=list(range(NCORES)), trace=trace
    )
    out = np.empty((B, D), np.float32)
    for i in range(NCORES):
        out[i * BC : (i + 1) * BC] = res.results[i]["outT"].T
    return out, res


def kernel(**inputs) -> np.ndarray:
    out, _ = run(inputs)
    return out
